# revision 16
# baseline (speedup 1.0000x reference)
"""Causal single-head attention (B=4, S=4096, D=1024) on 8 TRN2 NeuronCores.

Sharding: core = (batch b, half h).  Each core computes attention output for
2048 queries of one batch: query chunks {0,3,4,7} (h=0) or {1,2,5,6} (h=1) of
8x512, which balances causal work.  Each core projects K^T/V for its full
batch (Q projections zippered in between the chunks); K^T lives in SBUF as
four independently-gated fp16 tiles and V is streamed from a DRAM scratch on
the Scalar HWDGE queue.
Scores are computed in the S^T = [k, q] layout so no on-device transposes are
needed anywhere:
  K^T/Q^T/V projections:  psum = sum_d WT[d,:128].T @ x^T[d,:]      (fp16)
  scores^T[k,q]        :  psum = sum_o KT[o,k128].T @ QT[o,q512]    (fp16)
  P = exp(s*scale) * causal_mask   (mask = (iota_k - iota_q) <= a[slot,j])
  den[1,q]             :  ones[k,1].T @ P^T                         (fp16)
  ctx^T[o,q]           :  psum = sum_k V[k,o128].T @ P^T[k,q]       (fp16)
  out = ctx^T * (1/den)  broadcast via ones[1,128].T @ recip[1,q]
"""

import sys

for _p in ("/opt/trn_rl_repo",):
    if _p not in sys.path:
        sys.path.insert(0, _p)

import numpy as np

B, S, D = 4, 4096, 1024
P = 128
CH = 512                       # query chunk
NSLOT = 4                      # chunks per core
NQ = NSLOT * CH                # queries per core
NK = [8, 16, 24, 32]           # k-tiles per slot (uniform across cores)
SLOTBASE = [0, 8, 24, 48]      # amat column base per slot
CHUNKS_H = [[0, 3, 4, 7], [1, 2, 5, 6]]
SCALE = 1.0 / 32.0             # 1/sqrt(D)

_PROGRAM = None


def _build_program():
    import concourse.bass as bass
    import concourse.tile as tile
    import concourse.mybir as mybir
    from concourse import bacc
    from concourse.bass import ds, ts

    f32 = mybir.dt.float32
    f16 = mybir.dt.float16

    nc = bacc.Bacc(trn_type="TRN2", target_bir_lowering=False, debug=False,
                   num_devices=8)

    xT = nc.declare_dram_parameter("xT", [D, S], f16, isOutput=False)
    xqT = nc.declare_dram_parameter("xqT", [D, NQ], f16, isOutput=False)
    wqT = nc.declare_dram_parameter("wqT", [D, D], f16, isOutput=False)
    wkT = nc.declare_dram_parameter("wkT", [D, D], f16, isOutput=False)
    wvT = nc.declare_dram_parameter("wvT", [D, D], f16, isOutput=False)
    amat = nc.declare_dram_parameter("amat", [P, 80], f16, isOutput=False)
    dmat = nc.declare_dram_parameter("dmat", [P, CH], f16, isOutput=False)
    ones_k = nc.declare_dram_parameter("ones_k", [P, 1], f16, isOutput=False)
    ones_r = nc.declare_dram_parameter("ones_r", [1, P], f32, isOutput=False)
    outT = nc.declare_dram_parameter("outT", [D, NQ], f32, isOutput=True)

    H = S // 4  # 1024: columns per resident K^T piece
    vscr = nc.dram_tensor("v_scratch", [S, D], f16)

    Exp = mybir.ActivationFunctionType.Exp
    is_le = mybir.AluOpType.is_le
    mult = mybir.AluOpType.mult

    with tile.TileContext(nc, pool_alloc_mode="queue") as tc:
        with (
            tc.tile_pool(name="kt", bufs=1) as kt_pool,
            tc.tile_pool(name="qt", bufs=1) as qt_pool,
            tc.tile_pool(name="const", bufs=1) as const_pool,
        ):
            KTp = [
                kt_pool.tile([P, 8, H], f16, tag=f"kt{i}", name=f"KTp{i}")
                for i in range(4)
            ]
            QTs = [
                qt_pool.tile([P, 8, CH], f16, tag=f"qt{i}", name=f"QTs{i}")
                for i in range(NSLOT)
            ]
            dmat_sb = const_pool.tile([P, CH], f16, tag="dmat")
            amat_sb = const_pool.tile([P, 80], f16, tag="amat")
            ones_k_sb = const_pool.tile([P, 1], f16, tag="onesk")
            ones_r_sb = const_pool.tile([1, P], f32, tag="onesr")
            nc.sync.dma_start(out=dmat_sb[:], in_=dmat[:])
            nc.sync.dma_start(out=amat_sb[:], in_=amat[:])
            nc.sync.dma_start(out=ones_k_sb[:], in_=ones_k[:])
            nc.sync.dma_start(out=ones_r_sb[:], in_=ones_r[:])

            # ---------- Phase 0+1: local projections (K, V, Q zippered) ----
            with (
                tc.tile_pool(name="w0", bufs=1) as w_pool,
                tc.tile_pool(name="xc", bufs=3) as x_pool,
                tc.tile_pool(name="xq", bufs=2) as xq_pool,
                tc.tile_pool(name="vb", bufs=3) as vb_pool,
                tc.tile_pool(name="ps0", bufs=4, space="PSUM") as ps_pool,
            ):
                wk = w_pool.tile([P, 8, D], f16, tag="wk")
                wv = w_pool.tile([P, 8, D], f16, tag="wv")
                wq = w_pool.tile([P, 8, D], f16, tag="wq")
                for half in range(2):
                    nc.sync.dma_start(
                        out=wk[:, :, ds(half * CH, CH)],
                        in_=wkT[:, ds(half * CH, CH)].rearrange(
                            "(a p) o -> p a o", p=P
                        ),
                    )
                nc.sync.dma_start(
                    out=wv[:], in_=wvT[:].rearrange("(a p) o -> p a o", p=P)
                )
                nc.scalar.dma_start(
                    out=wq[:], in_=wqT[:].rearrange("(a p) o -> p a o", p=P)
                )

                def load_xq(c):
                    xq = xq_pool.tile([P, 8, CH], f16, tag="xq", name=f"xq{c}")
                    nc.scalar.dma_start(
                        out=xq[:],
                        in_=xqT[:, ts(c, CH)].rearrange("(a p) s -> p a s", p=P),
                    )
                    return xq

                xq_pending = [load_xq(0), load_xq(1)]

                def proj_q(slot):
                    xq = xq_pending[slot]
                    for o in range(8):
                        ps = ps_pool.tile([P, CH], f32, tag="ps", name="psq")
                        for d in range(8):
                            nc.tensor.matmul(
                                ps[:],
                                lhsT=wq[:, d, ts(o, P)],
                                rhs=xq[:, d, :],
                                start=(d == 0),
                                stop=(d == 7),
                            )
                        nc.vector.tensor_copy(QTs[slot][:, o, :], ps[:])

                for c in range(8):
                    xc = x_pool.tile([P, 8, CH], f16, tag="xc", name=f"xc{c}")
                    nc.sync.dma_start(
                        out=xc[:],
                        in_=xT[:, ts(c, CH)].rearrange("(a p) s -> p a s", p=P),
                    )
                    for o in range(8):
                        ps = ps_pool.tile([P, CH], f32, tag="ps", name="psk")
                        for d in range(8):
                            nc.tensor.matmul(
                                ps[:],
                                lhsT=wk[:, d, ts(o, P)],
                                rhs=xc[:, d, :],
                                start=(d == 0),
                                stop=(d == 7),
                            )
                        nc.vector.tensor_copy(
                            KTp[c // 2][:, o, ds((c % 2) * CH, CH)], ps[:]
                        )
                    for kt_i in range(4):
                        vb = vb_pool.tile([P, D], f16, tag="vb", name="vb")
                        for oh in range(2):
                            ps = ps_pool.tile([P, CH], f32, tag="ps", name="psv")
                            for d in range(8):
                                nc.tensor.matmul(
                                    ps[:],
                                    lhsT=xc[:, d, ts(kt_i, P)],
                                    rhs=wv[:, d, ts(oh, CH)],
                                    start=(d == 0),
                                    stop=(d == 7),
                                )
                            nc.scalar.copy(vb[:, ts(oh, CH)], ps[:])
                        nc.sync.dma_start(
                            out=vscr[ds(c * CH + kt_i * P, P), :], in_=vb[:]
                        )
                    if c < 4:
                        proj_q(c)
                        if c < 2:
                            xq_pending.append(load_xq(c + 2))

            # ---------------- Phase 2: attention ---------------------------
            with (
                tc.tile_pool(name="ctx", bufs=2) as ctx_pool,
                tc.tile_pool(name="vt", bufs=8) as v_pool,
                tc.tile_pool(name="pt", bufs=10) as p_pool,
                tc.tile_pool(name="et", bufs=3) as e_pool,
                tc.tile_pool(name="fo", bufs=3) as f_pool,
                tc.tile_pool(name="dsb", bufs=2) as den_pool,
                tc.tile_pool(name="pss", bufs=3, space="PSUM") as s_ps_pool,
                tc.tile_pool(name="psc", bufs=2, space="PSUM") as c_ps_pool,
                tc.tile_pool(name="psd", bufs=2, space="PSUM") as d_ps_pool,
                tc.tile_pool(name="psb", bufs=1, space="PSUM") as b_ps_pool,
            ):
                for slot in range(NSLOT):
                    nk = NK[slot]
                    ctx = ctx_pool.tile([P, 8, CH], f32, tag="ctx", name="ctx")
                    den = den_pool.tile([1, CH], f32, tag="den", name="den")
                    for blk in range(nk // 4):
                        p_tiles = []
                        v_tiles = []
                        for j4 in range(4):
                            j = blk * 4 + j4
                            vt = v_pool.tile([P, D], f16, tag="vt", name="vt")
                            nc.scalar.dma_start(out=vt[:], in_=vscr[ds(j * P, P), :])
                            sps = s_ps_pool.tile([P, CH], f32, name="sps")
                            for o in range(8):
                                nc.tensor.matmul(
                                    sps[:],
                                    lhsT=KTp[j // 8][:, o, ds((j % 8) * P, P)],
                                    rhs=QTs[slot][:, o, :],
                                    start=(o == 0),
                                    stop=(o == 7),
                                )
                            et = e_pool.tile([P, CH], f16, tag="et", name="et")
                            nc.scalar.activation(et[:], sps[:], Exp, scale=SCALE)
                            pt = p_pool.tile([P, CH], f16, tag="pt", name="pt")
                            col = SLOTBASE[slot] + j
                            nc.vector.scalar_tensor_tensor(
                                out=pt[:],
                                in0=dmat_sb[:],
                                scalar=amat_sb[:, ds(col, 1)],
                                in1=et[:],
                                op0=is_le,
                                op1=mult,
                            )
                            p_tiles.append(pt)
                            v_tiles.append(vt)
                        dps = d_ps_pool.tile([1, CH], f32, name="dps")
                        for j4 in range(4):
                            nc.tensor.matmul(
                                dps[:],
                                lhsT=ones_k_sb[:],
                                rhs=p_tiles[j4][:],
                                start=(j4 == 0),
                                stop=(j4 == 3),
                            )
                        if blk == 0:
                            nc.vector.tensor_copy(den[:], dps[:])
                        else:
                            nc.vector.tensor_add(den[:], den[:], dps[:])
                        for o in range(8):
                            cps = c_ps_pool.tile([P, CH], f32, name="cps")
                            for j4 in range(4):
                                nc.tensor.matmul(
                                    cps[:],
                                    lhsT=v_tiles[j4][:, ts(o, P)],
                                    rhs=p_tiles[j4][:],
                                    start=(j4 == 0),
                                    stop=(j4 == 3),
                                )
                            if blk == 0:
                                nc.vector.tensor_copy(ctx[:, o, :], cps[:])
                            else:
                                nc.vector.tensor_add(
                                    ctx[:, o, :], ctx[:, o, :], cps[:]
                                )
                    rec = den_pool.tile([1, CH], f32, tag="rec", name="rec")
                    nc.vector.reciprocal(rec[:], den[:])
                    bps = b_ps_pool.tile([P, CH], f32, name="bps")
                    nc.tensor.matmul(
                        bps[:], lhsT=ones_r_sb[:], rhs=rec[:], start=True, stop=True
                    )
                    for o in range(8):
                        ft = f_pool.tile([P, CH], f32, tag="ft", name="ft")
                        nc.vector.tensor_mul(ft[:], ctx[:, o, :], bps[:])
                        nc.sync.dma_start(
                            out=outT[ds(o * P, P), ts(slot, CH)], in_=ft[:]
                        )

    nc.compile()
    return nc


def _get_program():
    global _PROGRAM
    if _PROGRAM is None:
        _PROGRAM = _build_program()
    return _PROGRAM


def _make_in_maps(x, W_query, W_key, W_value):
    xT = np.ascontiguousarray(
        np.asarray(x, dtype=np.float32).transpose(0, 2, 1).astype(np.float16)
    )
    wqT = np.ascontiguousarray(np.asarray(W_query, dtype=np.float32).T.astype(np.float16))
    wkT = np.ascontiguousarray(np.asarray(W_key, dtype=np.float32).T.astype(np.float16))
    wvT = np.ascontiguousarray(np.asarray(W_value, dtype=np.float32).T.astype(np.float16))
    dmat = (
        np.arange(P, dtype=np.float32)[:, None] - np.arange(CH, dtype=np.float32)[None, :]
    )
    dmat = np.ascontiguousarray(dmat.astype(np.float16))
    amat_h = []
    for h in range(2):
        a = np.zeros((P, 80), np.float16)
        for slot in range(NSLOT):
            cid = CHUNKS_H[h][slot]
            for j in range(NK[slot]):
                a[:, SLOTBASE[slot] + j] = CH * cid - P * j
        amat_h.append(a)
    ones_k = np.ones((P, 1), np.float16)
    ones_r = np.ones((1, P), np.float32)

    in_maps = []
    for core in range(8):
        b, h = core // 2, core % 2
        xq_cols = np.concatenate(
            [np.arange(c * CH, (c + 1) * CH) for c in CHUNKS_H[h]]
        )
        xqT_b = np.ascontiguousarray(xT[b][:, xq_cols])
        in_maps.append(
            {
                "xT": xT[b],
                "xqT": xqT_b,
                "wqT": wqT,
                "wkT": wkT,
                "wvT": wvT,
                "amat": amat_h[h],
                "dmat": dmat,
                "ones_k": ones_k,
                "ones_r": ones_r,
            }
        )
    return in_maps


def _assemble(results):
    out = np.empty((B, S, D), np.float32)
    for core in range(8):
        b, h = core // 2, core % 2
        oT = np.asarray(results[core]["outT"])  # [D, NQ]
        for slot, c in enumerate(CHUNKS_H[h]):
            out[b, c * CH : (c + 1) * CH, :] = oT[:, slot * CH : (slot + 1) * CH].T
    return out


def run(inputs, trace=False, trace_cores=None):
    """Run the kernel; returns (output, BassKernelResults)."""
    from concourse.bass_utils import run_bass_kernel_spmd

    nc = _get_program()
    in_maps = _make_in_maps(
        inputs["x"], inputs["W_query"], inputs["W_key"], inputs["W_value"]
    )
    kw = {}
    if trace:
        kw = dict(trace=True, trace_cores=trace_cores, stitch_traces=False)
    res = run_bass_kernel_spmd(nc, in_maps, list(range(8)), **kw)
    return _assemble(res.results), res


def kernel(x, W_query, W_key, W_value):
    out, _ = run({"x": x, "W_query": W_query, "W_key": W_key, "W_value": W_value})
    return out


# revision 17
# speedup vs baseline: 1.0013x; 1.0013x over previous
"""Causal single-head attention (B=4, S=4096, D=1024) on 8 TRN2 NeuronCores.

Sharding: core = (batch b, half h).  Each core computes attention output for
2048 queries of one batch: query chunks {0,3,4,7} (h=0) or {1,2,5,6} (h=1) of
8x512, which balances causal work.  Each core projects K^T/V for its full
batch (Q projections zippered in between the chunks); K^T lives in SBUF as
four independently-gated fp16 tiles and V is streamed from a DRAM scratch on
the Scalar HWDGE queue.
Scores are computed in the S^T = [k, q] layout so no on-device transposes are
needed anywhere:
  K^T/Q^T/V projections:  psum = sum_d WT[d,:128].T @ x^T[d,:]      (fp16)
  scores^T[k,q]        :  psum = sum_o KT[o,k128].T @ QT[o,q512]    (fp16)
  P = exp(s*scale) * causal_mask   (mask = (iota_k - iota_q) <= a[slot,j])
  den[1,q]             :  ones[k,1].T @ P^T                         (fp16)
  ctx^T[o,q]           :  psum = sum_k V[k,o128].T @ P^T[k,q]       (fp16)
  out = ctx^T * (1/den)  broadcast via ones[1,128].T @ recip[1,q]
"""

import sys

for _p in ("/opt/trn_rl_repo",):
    if _p not in sys.path:
        sys.path.insert(0, _p)

import numpy as np

B, S, D = 4, 4096, 1024
P = 128
CH = 512                       # query chunk
NSLOT = 4                      # chunks per core
NQ = NSLOT * CH                # queries per core
NK = [8, 16, 24, 32]           # k-tiles per slot (uniform across cores)
SLOTBASE = [0, 8, 24, 48]      # amat column base per slot
CHUNKS_H = [[0, 3, 4, 7], [1, 2, 5, 6]]
SCALE = 1.0 / 32.0             # 1/sqrt(D)

_PROGRAM = None


def _build_program():
    import concourse.bass as bass
    import concourse.tile as tile
    import concourse.mybir as mybir
    from concourse import bacc
    from concourse.bass import ds, ts

    f32 = mybir.dt.float32
    f16 = mybir.dt.float16

    nc = bacc.Bacc(trn_type="TRN2", target_bir_lowering=False, debug=False,
                   num_devices=8)

    xT = nc.declare_dram_parameter("xT", [D, S], f16, isOutput=False)
    xqT = nc.declare_dram_parameter("xqT", [D, NQ], f16, isOutput=False)
    wqT = nc.declare_dram_parameter("wqT", [D, D], f16, isOutput=False)
    wkT = nc.declare_dram_parameter("wkT", [D, D], f16, isOutput=False)
    wvT = nc.declare_dram_parameter("wvT", [D, D], f16, isOutput=False)
    amat = nc.declare_dram_parameter("amat", [P, 80], f16, isOutput=False)
    dmat = nc.declare_dram_parameter("dmat", [P, CH], f16, isOutput=False)
    ones_k = nc.declare_dram_parameter("ones_k", [P, 1], f16, isOutput=False)
    ones_r = nc.declare_dram_parameter("ones_r", [1, P], f32, isOutput=False)
    outT = nc.declare_dram_parameter("outT", [D, NQ], f32, isOutput=True)

    H = S // 4  # 1024: columns per resident K^T piece
    vscr = nc.dram_tensor("v_scratch", [S, D], f16)

    Exp = mybir.ActivationFunctionType.Exp
    is_le = mybir.AluOpType.is_le
    mult = mybir.AluOpType.mult

    with tile.TileContext(nc, pool_alloc_mode="queue") as tc:
        with (
            tc.tile_pool(name="kt", bufs=1) as kt_pool,
            tc.tile_pool(name="qt", bufs=1) as qt_pool,
            tc.tile_pool(name="const", bufs=1) as const_pool,
        ):
            KTp = [
                kt_pool.tile([P, 8, H], f16, tag=f"kt{i}", name=f"KTp{i}")
                for i in range(4)
            ]
            QTs = [
                qt_pool.tile([P, 8, CH], f16, tag=f"qt{i}", name=f"QTs{i}")
                for i in range(NSLOT)
            ]
            dmat_sb = const_pool.tile([P, CH], f16, tag="dmat")
            amat_sb = const_pool.tile([P, 80], f16, tag="amat")
            ones_k_sb = const_pool.tile([P, 1], f16, tag="onesk")
            ones_r_sb = const_pool.tile([1, P], f32, tag="onesr")
            nc.sync.dma_start(out=dmat_sb[:], in_=dmat[:])
            nc.sync.dma_start(out=amat_sb[:], in_=amat[:])
            nc.sync.dma_start(out=ones_k_sb[:], in_=ones_k[:])
            nc.sync.dma_start(out=ones_r_sb[:], in_=ones_r[:])

            # ---------- Phase 0+1: local projections (K, V, Q zippered) ----
            with (
                tc.tile_pool(name="w0", bufs=1) as w_pool,
                tc.tile_pool(name="xc", bufs=3) as x_pool,
                tc.tile_pool(name="xq", bufs=3) as xq_pool,
                tc.tile_pool(name="vb", bufs=3) as vb_pool,
                tc.tile_pool(name="ps0", bufs=4, space="PSUM") as ps_pool,
            ):
                wk = w_pool.tile([P, 8, D], f16, tag="wk")
                wv = w_pool.tile([P, 8, D], f16, tag="wv")
                wq = w_pool.tile([P, 8, D], f16, tag="wq")
                for half in range(2):
                    nc.sync.dma_start(
                        out=wk[:, :, ds(half * CH, CH)],
                        in_=wkT[:, ds(half * CH, CH)].rearrange(
                            "(a p) o -> p a o", p=P
                        ),
                    )

                def load_xq(c):
                    xq = xq_pool.tile([P, 8, CH], f16, tag="xq", name=f"xq{c}")
                    nc.scalar.dma_start(
                        out=xq[:],
                        in_=xqT[:, ts(c, CH)].rearrange("(a p) s -> p a s", p=P),
                    )
                    return xq

                xq_pending = []

                def proj_q(slot):
                    xq = xq_pending[slot]
                    for o in range(8):
                        ps = ps_pool.tile([P, CH], f32, tag="ps", name="psq")
                        for d in range(8):
                            nc.tensor.matmul(
                                ps[:],
                                lhsT=wq[:, d, ts(o, P)],
                                rhs=xq[:, d, :],
                                start=(d == 0),
                                stop=(d == 7),
                            )
                        nc.vector.tensor_copy(QTs[slot][:, o, :], ps[:])

                for c in range(8):
                    xc = x_pool.tile([P, 8, CH], f16, tag="xc", name=f"xc{c}")
                    nc.sync.dma_start(
                        out=xc[:],
                        in_=xT[:, ts(c, CH)].rearrange("(a p) s -> p a s", p=P),
                    )
                    for o in range(8):
                        ps = ps_pool.tile([P, CH], f32, tag="ps", name="psk")
                        for d in range(8):
                            nc.tensor.matmul(
                                ps[:],
                                lhsT=wk[:, d, ts(o, P)],
                                rhs=xc[:, d, :],
                                start=(d == 0),
                                stop=(d == 7),
                            )
                        nc.vector.tensor_copy(
                            KTp[c // 2][:, o, ds((c % 2) * CH, CH)], ps[:]
                        )
                    if c == 0:
                        # deferred loads: SP/ACT reach these only after the
                        # first chunk's copies, leaving full DMA bandwidth to
                        # the critical wk+xc0 at kernel start
                        nc.sync.dma_start(
                            out=wv[:], in_=wvT[:].rearrange("(a p) o -> p a o", p=P)
                        )
                        nc.scalar.dma_start(
                            out=wq[:], in_=wqT[:].rearrange("(a p) o -> p a o", p=P)
                        )
                        xq_pending.append(load_xq(0))
                        xq_pending.append(load_xq(1))
                    for kt_i in range(4):
                        vb = vb_pool.tile([P, D], f16, tag="vb", name="vb")
                        for oh in range(2):
                            ps = ps_pool.tile([P, CH], f32, tag="ps", name="psv")
                            for d in range(8):
                                nc.tensor.matmul(
                                    ps[:],
                                    lhsT=xc[:, d, ts(kt_i, P)],
                                    rhs=wv[:, d, ts(oh, CH)],
                                    start=(d == 0),
                                    stop=(d == 7),
                                )
                            nc.scalar.copy(vb[:, ts(oh, CH)], ps[:])
                        nc.sync.dma_start(
                            out=vscr[ds(c * CH + kt_i * P, P), :], in_=vb[:]
                        )
                    if 1 <= c <= 4:
                        proj_q(c - 1)
                        if c <= 2:
                            xq_pending.append(load_xq(c + 1))

            # ---------------- Phase 2: attention ---------------------------
            with (
                tc.tile_pool(name="ctx", bufs=2) as ctx_pool,
                tc.tile_pool(name="vt", bufs=8) as v_pool,
                tc.tile_pool(name="pt", bufs=10) as p_pool,
                tc.tile_pool(name="et", bufs=3) as e_pool,
                tc.tile_pool(name="fo", bufs=3) as f_pool,
                tc.tile_pool(name="dsb", bufs=2) as den_pool,
                tc.tile_pool(name="pss", bufs=3, space="PSUM") as s_ps_pool,
                tc.tile_pool(name="psc", bufs=2, space="PSUM") as c_ps_pool,
                tc.tile_pool(name="psd", bufs=2, space="PSUM") as d_ps_pool,
                tc.tile_pool(name="psb", bufs=1, space="PSUM") as b_ps_pool,
            ):
                for slot in range(NSLOT):
                    nk = NK[slot]
                    ctx = ctx_pool.tile([P, 8, CH], f32, tag="ctx", name="ctx")
                    den = den_pool.tile([1, CH], f32, tag="den", name="den")
                    for blk in range(nk // 4):
                        p_tiles = []
                        v_tiles = []
                        for j4 in range(4):
                            j = blk * 4 + j4
                            vt = v_pool.tile([P, D], f16, tag="vt", name="vt")
                            nc.scalar.dma_start(out=vt[:], in_=vscr[ds(j * P, P), :])
                            sps = s_ps_pool.tile([P, CH], f32, name="sps")
                            for o in range(8):
                                nc.tensor.matmul(
                                    sps[:],
                                    lhsT=KTp[j // 8][:, o, ds((j % 8) * P, P)],
                                    rhs=QTs[slot][:, o, :],
                                    start=(o == 0),
                                    stop=(o == 7),
                                )
                            et = e_pool.tile([P, CH], f16, tag="et", name="et")
                            nc.scalar.activation(et[:], sps[:], Exp, scale=SCALE)
                            pt = p_pool.tile([P, CH], f16, tag="pt", name="pt")
                            col = SLOTBASE[slot] + j
                            nc.vector.scalar_tensor_tensor(
                                out=pt[:],
                                in0=dmat_sb[:],
                                scalar=amat_sb[:, ds(col, 1)],
                                in1=et[:],
                                op0=is_le,
                                op1=mult,
                            )
                            p_tiles.append(pt)
                            v_tiles.append(vt)
                        dps = d_ps_pool.tile([1, CH], f32, name="dps")
                        for j4 in range(4):
                            nc.tensor.matmul(
                                dps[:],
                                lhsT=ones_k_sb[:],
                                rhs=p_tiles[j4][:],
                                start=(j4 == 0),
                                stop=(j4 == 3),
                            )
                        if blk == 0:
                            nc.vector.tensor_copy(den[:], dps[:])
                        else:
                            nc.vector.tensor_add(den[:], den[:], dps[:])
                        for o in range(8):
                            cps = c_ps_pool.tile([P, CH], f32, name="cps")
                            for j4 in range(4):
                                nc.tensor.matmul(
                                    cps[:],
                                    lhsT=v_tiles[j4][:, ts(o, P)],
                                    rhs=p_tiles[j4][:],
                                    start=(j4 == 0),
                                    stop=(j4 == 3),
                                )
                            if blk == 0:
                                nc.vector.tensor_copy(ctx[:, o, :], cps[:])
                            else:
                                nc.vector.tensor_add(
                                    ctx[:, o, :], ctx[:, o, :], cps[:]
                                )
                    bps = b_ps_pool.tile([P, CH], f32, name="bps")
                    nc.tensor.matmul(
                        bps[:], lhsT=ones_r_sb[:], rhs=den[:], start=True, stop=True
                    )
                    rec = f_pool.tile([P, CH], f32, tag="rec", name="rec")
                    nc.vector.reciprocal(rec[:], bps[:])
                    for o in range(8):
                        ft = f_pool.tile([P, CH], f32, tag="ft", name="ft")
                        nc.vector.tensor_mul(ft[:], ctx[:, o, :], rec[:])
                        nc.sync.dma_start(
                            out=outT[ds(o * P, P), ts(slot, CH)], in_=ft[:]
                        )

    nc.compile()
    return nc


def _get_program():
    global _PROGRAM
    if _PROGRAM is None:
        _PROGRAM = _build_program()
    return _PROGRAM


def _make_in_maps(x, W_query, W_key, W_value):
    xT = np.ascontiguousarray(
        np.asarray(x, dtype=np.float32).transpose(0, 2, 1).astype(np.float16)
    )
    wqT = np.ascontiguousarray(np.asarray(W_query, dtype=np.float32).T.astype(np.float16))
    wkT = np.ascontiguousarray(np.asarray(W_key, dtype=np.float32).T.astype(np.float16))
    wvT = np.ascontiguousarray(np.asarray(W_value, dtype=np.float32).T.astype(np.float16))
    dmat = (
        np.arange(P, dtype=np.float32)[:, None] - np.arange(CH, dtype=np.float32)[None, :]
    )
    dmat = np.ascontiguousarray(dmat.astype(np.float16))
    amat_h = []
    for h in range(2):
        a = np.zeros((P, 80), np.float16)
        for slot in range(NSLOT):
            cid = CHUNKS_H[h][slot]
            for j in range(NK[slot]):
                a[:, SLOTBASE[slot] + j] = CH * cid - P * j
        amat_h.append(a)
    ones_k = np.ones((P, 1), np.float16)
    ones_r = np.ones((1, P), np.float32)

    in_maps = []
    for core in range(8):
        b, h = core // 2, core % 2
        xq_cols = np.concatenate(
            [np.arange(c * CH, (c + 1) * CH) for c in CHUNKS_H[h]]
        )
        xqT_b = np.ascontiguousarray(xT[b][:, xq_cols])
        in_maps.append(
            {
                "xT": xT[b],
                "xqT": xqT_b,
                "wqT": wqT,
                "wkT": wkT,
                "wvT": wvT,
                "amat": amat_h[h],
                "dmat": dmat,
                "ones_k": ones_k,
                "ones_r": ones_r,
            }
        )
    return in_maps


def _assemble(results):
    out = np.empty((B, S, D), np.float32)
    for core in range(8):
        b, h = core // 2, core % 2
        oT = np.asarray(results[core]["outT"])  # [D, NQ]
        for slot, c in enumerate(CHUNKS_H[h]):
            out[b, c * CH : (c + 1) * CH, :] = oT[:, slot * CH : (slot + 1) * CH].T
    return out


def run(inputs, trace=False, trace_cores=None):
    """Run the kernel; returns (output, BassKernelResults)."""
    from concourse.bass_utils import run_bass_kernel_spmd

    nc = _get_program()
    in_maps = _make_in_maps(
        inputs["x"], inputs["W_query"], inputs["W_key"], inputs["W_value"]
    )
    kw = {}
    if trace:
        kw = dict(trace=True, trace_cores=trace_cores, stitch_traces=False)
    res = run_bass_kernel_spmd(nc, in_maps, list(range(8)), **kw)
    return _assemble(res.results), res


def kernel(x, W_query, W_key, W_value):
    out, _ = run({"x": x, "W_query": W_query, "W_key": W_key, "W_value": W_value})
    return out


# revision 18
# speedup vs baseline: 1.0105x; 1.0093x over previous
"""Causal single-head attention (B=4, S=4096, D=1024) on 8 TRN2 NeuronCores.

Sharding: core = (batch b, half h).  Each core computes attention output for
2048 queries of one batch: query chunks {0,3,4,7} (h=0) or {1,2,5,6} (h=1) of
8x512, which balances causal work.  Each core projects K^T/V for its full
batch (Q projections zippered in between the chunks); K^T lives in SBUF as
four independently-gated fp16 tiles and V is streamed from a DRAM scratch on
the Scalar HWDGE queue.
Scores are computed in the S^T = [k, q] layout so no on-device transposes are
needed anywhere:
  K^T/Q^T/V projections:  psum = sum_d WT[d,:128].T @ x^T[d,:]      (fp16)
  scores^T[k,q]        :  psum = sum_o KT[o,k128].T @ QT[o,q512]    (fp16)
  P = exp(s*scale) * causal_mask   (mask = (iota_k - iota_q) <= a[slot,j])
  den[1,q]             :  ones[k,1].T @ P^T                         (fp16)
  ctx^T[o,q]           :  psum = sum_k V[k,o128].T @ P^T[k,q]       (fp16)
  out = ctx^T * (1/den)  broadcast via ones[1,128].T @ recip[1,q]
"""

import sys

for _p in ("/opt/trn_rl_repo",):
    if _p not in sys.path:
        sys.path.insert(0, _p)

import numpy as np

B, S, D = 4, 4096, 1024
P = 128
CH = 512                       # query chunk
NSLOT = 4                      # chunks per core
NQ = NSLOT * CH                # queries per core
NK = [8, 16, 24, 32]           # k-tiles per slot (uniform across cores)
SLOTBASE = [0, 8, 24, 48]      # amat column base per slot
CHUNKS_H = [[0, 3, 4, 7], [1, 2, 5, 6]]
SCALE = 1.0 / 32.0             # 1/sqrt(D)

_PROGRAM = None


def _build_program():
    import concourse.bass as bass
    import concourse.tile as tile
    import concourse.mybir as mybir
    from concourse import bacc
    from concourse.bass import ds, ts

    f32 = mybir.dt.float32
    f16 = mybir.dt.float16

    nc = bacc.Bacc(trn_type="TRN2", target_bir_lowering=False, debug=False,
                   num_devices=8)

    xT = nc.declare_dram_parameter("xT", [8, P, 8, CH], f16, isOutput=False)
    xqT = nc.declare_dram_parameter("xqT", [NSLOT, P, 8, CH], f16, isOutput=False)
    wqT = nc.declare_dram_parameter("wqT", [P, 8, D], f16, isOutput=False)
    wkT = nc.declare_dram_parameter("wkT", [P, 8, D], f16, isOutput=False)
    wvT = nc.declare_dram_parameter("wvT", [P, 8, D], f16, isOutput=False)
    amat = nc.declare_dram_parameter("amat", [P, 80], f16, isOutput=False)
    dmat = nc.declare_dram_parameter("dmat", [P, CH], f16, isOutput=False)
    ones_k = nc.declare_dram_parameter("ones_k", [P, 1], f16, isOutput=False)
    ones_r = nc.declare_dram_parameter("ones_r", [1, P], f32, isOutput=False)
    outT = nc.declare_dram_parameter("outT", [D, NQ], f32, isOutput=True)

    H = S // 4  # 1024: columns per resident K^T piece
    vscr = nc.dram_tensor("v_scratch", [S, D], f16)

    Exp = mybir.ActivationFunctionType.Exp
    is_le = mybir.AluOpType.is_le
    mult = mybir.AluOpType.mult

    with tile.TileContext(nc, pool_alloc_mode="queue") as tc:
        with (
            tc.tile_pool(name="kt", bufs=1) as kt_pool,
            tc.tile_pool(name="qt", bufs=1) as qt_pool,
            tc.tile_pool(name="const", bufs=1) as const_pool,
        ):
            KTp = [
                kt_pool.tile([P, 8, H], f16, tag=f"kt{i}", name=f"KTp{i}")
                for i in range(4)
            ]
            QTs = [
                qt_pool.tile([P, 8, CH], f16, tag=f"qt{i}", name=f"QTs{i}")
                for i in range(NSLOT)
            ]
            dmat_sb = const_pool.tile([P, CH], f16, tag="dmat")
            amat_sb = const_pool.tile([P, 80], f16, tag="amat")
            ones_k_sb = const_pool.tile([P, 1], f16, tag="onesk")
            ones_r_sb = const_pool.tile([1, P], f32, tag="onesr")
            nc.sync.dma_start(out=dmat_sb[:], in_=dmat[:])
            nc.sync.dma_start(out=amat_sb[:], in_=amat[:])
            nc.sync.dma_start(out=ones_k_sb[:], in_=ones_k[:])
            nc.sync.dma_start(out=ones_r_sb[:], in_=ones_r[:])

            # ---------- Phase 0+1: local projections (K, V, Q zippered) ----
            with (
                tc.tile_pool(name="w0", bufs=1) as w_pool,
                tc.tile_pool(name="xc", bufs=3) as x_pool,
                tc.tile_pool(name="xq", bufs=3) as xq_pool,
                tc.tile_pool(name="vb", bufs=3) as vb_pool,
                tc.tile_pool(name="ps0", bufs=4, space="PSUM") as ps_pool,
            ):
                wk = w_pool.tile([P, 8, D], f16, tag="wk")
                wv = w_pool.tile([P, 8, D], f16, tag="wv")
                wq = w_pool.tile([P, 8, D], f16, tag="wq")
                for half in range(2):
                    nc.sync.dma_start(
                        out=wk[:, :, ds(half * CH, CH)],
                        in_=wkT[:, :, ds(half * CH, CH)],
                    )

                def load_xq(c):
                    xq = xq_pool.tile([P, 8, CH], f16, tag="xq", name=f"xq{c}")
                    nc.scalar.dma_start(
                        out=xq[:],
                        in_=xqT[c],
                    )
                    return xq

                xq_pending = []

                def proj_q(slot):
                    xq = xq_pending[slot]
                    for o in range(8):
                        ps = ps_pool.tile([P, CH], f32, tag="ps", name="psq")
                        for d in range(8):
                            nc.tensor.matmul(
                                ps[:],
                                lhsT=wq[:, d, ts(o, P)],
                                rhs=xq[:, d, :],
                                start=(d == 0),
                                stop=(d == 7),
                            )
                        nc.vector.tensor_copy(QTs[slot][:, o, :], ps[:])

                for c in range(8):
                    xc = x_pool.tile([P, 8, CH], f16, tag="xc", name=f"xc{c}")
                    nc.sync.dma_start(
                        out=xc[:],
                        in_=xT[c],
                    )
                    for o in range(8):
                        ps = ps_pool.tile([P, CH], f32, tag="ps", name="psk")
                        for d in range(8):
                            nc.tensor.matmul(
                                ps[:],
                                lhsT=wk[:, d, ts(o, P)],
                                rhs=xc[:, d, :],
                                start=(d == 0),
                                stop=(d == 7),
                            )
                        nc.vector.tensor_copy(
                            KTp[c // 2][:, o, ds((c % 2) * CH, CH)], ps[:]
                        )
                    if c == 0:
                        # deferred loads: SP/ACT reach these only after the
                        # first chunk's copies, leaving full DMA bandwidth to
                        # the critical wk+xc0 at kernel start
                        nc.sync.dma_start(
                            out=wv[:], in_=wvT[:]
                        )
                        nc.scalar.dma_start(
                            out=wq[:], in_=wqT[:]
                        )
                        xq_pending.append(load_xq(0))
                        xq_pending.append(load_xq(1))
                    for kt_i in range(4):
                        vb = vb_pool.tile([P, D], f16, tag="vb", name="vb")
                        for oh in range(2):
                            ps = ps_pool.tile([P, CH], f32, tag="ps", name="psv")
                            for d in range(8):
                                nc.tensor.matmul(
                                    ps[:],
                                    lhsT=xc[:, d, ts(kt_i, P)],
                                    rhs=wv[:, d, ts(oh, CH)],
                                    start=(d == 0),
                                    stop=(d == 7),
                                )
                            nc.scalar.copy(vb[:, ts(oh, CH)], ps[:])
                        nc.sync.dma_start(
                            out=vscr[ds(c * CH + kt_i * P, P), :], in_=vb[:]
                        )
                    if 1 <= c <= 4:
                        proj_q(c - 1)
                        if c <= 2:
                            xq_pending.append(load_xq(c + 1))

            # ---------------- Phase 2: attention ---------------------------
            with (
                tc.tile_pool(name="ctx", bufs=2) as ctx_pool,
                tc.tile_pool(name="vt", bufs=8) as v_pool,
                tc.tile_pool(name="pt", bufs=10) as p_pool,
                tc.tile_pool(name="et", bufs=3) as e_pool,
                tc.tile_pool(name="fo", bufs=3) as f_pool,
                tc.tile_pool(name="dsb", bufs=2) as den_pool,
                tc.tile_pool(name="pss", bufs=3, space="PSUM") as s_ps_pool,
                tc.tile_pool(name="psc", bufs=2, space="PSUM") as c_ps_pool,
                tc.tile_pool(name="psd", bufs=2, space="PSUM") as d_ps_pool,
                tc.tile_pool(name="psb", bufs=1, space="PSUM") as b_ps_pool,
            ):
                for slot in range(NSLOT):
                    nk = NK[slot]
                    ctx = ctx_pool.tile([P, 8, CH], f32, tag="ctx", name="ctx")
                    den = den_pool.tile([1, CH], f32, tag="den", name="den")
                    for blk in range(nk // 4):
                        p_tiles = []
                        v_tiles = []
                        for j4 in range(4):
                            j = blk * 4 + j4
                            vt = v_pool.tile([P, D], f16, tag="vt", name="vt")
                            nc.scalar.dma_start(out=vt[:], in_=vscr[ds(j * P, P), :])
                            sps = s_ps_pool.tile([P, CH], f32, name="sps")
                            for o in range(8):
                                nc.tensor.matmul(
                                    sps[:],
                                    lhsT=KTp[j // 8][:, o, ds((j % 8) * P, P)],
                                    rhs=QTs[slot][:, o, :],
                                    start=(o == 0),
                                    stop=(o == 7),
                                )
                            et = e_pool.tile([P, CH], f16, tag="et", name="et")
                            nc.scalar.activation(et[:], sps[:], Exp, scale=SCALE)
                            pt = p_pool.tile([P, CH], f16, tag="pt", name="pt")
                            col = SLOTBASE[slot] + j
                            nc.vector.scalar_tensor_tensor(
                                out=pt[:],
                                in0=dmat_sb[:],
                                scalar=amat_sb[:, ds(col, 1)],
                                in1=et[:],
                                op0=is_le,
                                op1=mult,
                            )
                            p_tiles.append(pt)
                            v_tiles.append(vt)
                        dps = d_ps_pool.tile([1, CH], f32, name="dps")
                        for j4 in range(4):
                            nc.tensor.matmul(
                                dps[:],
                                lhsT=ones_k_sb[:],
                                rhs=p_tiles[j4][:],
                                start=(j4 == 0),
                                stop=(j4 == 3),
                            )
                        if blk == 0:
                            nc.vector.tensor_copy(den[:], dps[:])
                        else:
                            nc.vector.tensor_add(den[:], den[:], dps[:])
                        for o in range(8):
                            cps = c_ps_pool.tile([P, CH], f32, name="cps")
                            for j4 in range(4):
                                nc.tensor.matmul(
                                    cps[:],
                                    lhsT=v_tiles[j4][:, ts(o, P)],
                                    rhs=p_tiles[j4][:],
                                    start=(j4 == 0),
                                    stop=(j4 == 3),
                                )
                            if blk == 0:
                                nc.vector.tensor_copy(ctx[:, o, :], cps[:])
                            else:
                                nc.vector.tensor_add(
                                    ctx[:, o, :], ctx[:, o, :], cps[:]
                                )
                    bps = b_ps_pool.tile([P, CH], f32, name="bps")
                    nc.tensor.matmul(
                        bps[:], lhsT=ones_r_sb[:], rhs=den[:], start=True, stop=True
                    )
                    rec = f_pool.tile([P, CH], f32, tag="rec", name="rec")
                    nc.vector.reciprocal(rec[:], bps[:])
                    for o in range(8):
                        ft = f_pool.tile([P, CH], f32, tag="ft", name="ft")
                        nc.vector.tensor_mul(ft[:], ctx[:, o, :], rec[:])
                        nc.sync.dma_start(
                            out=outT[ds(o * P, P), ts(slot, CH)], in_=ft[:]
                        )

    nc.compile()
    return nc


def _get_program():
    global _PROGRAM
    if _PROGRAM is None:
        _PROGRAM = _build_program()
    return _PROGRAM


def _make_in_maps(x, W_query, W_key, W_value):
    xT = np.ascontiguousarray(
        np.asarray(x, dtype=np.float32).transpose(0, 2, 1).astype(np.float16)
    )

    def tile_w(w):
        # [d, o] -> [p, d_slab, o]
        wt = np.asarray(w, dtype=np.float32).T.astype(np.float16)
        return np.ascontiguousarray(wt.reshape(8, P, D).transpose(1, 0, 2))

    def tile_x(xt, nch):
        # [d, s] -> [chunk, p, d_slab, s_off]
        return np.ascontiguousarray(
            xt.reshape(8, P, nch, CH).transpose(2, 1, 0, 3)
        )

    wqT = tile_w(W_query)
    wkT = tile_w(W_key)
    wvT = tile_w(W_value)
    dmat = (
        np.arange(P, dtype=np.float32)[:, None] - np.arange(CH, dtype=np.float32)[None, :]
    )
    dmat = np.ascontiguousarray(dmat.astype(np.float16))
    amat_h = []
    for h in range(2):
        a = np.zeros((P, 80), np.float16)
        for slot in range(NSLOT):
            cid = CHUNKS_H[h][slot]
            for j in range(NK[slot]):
                a[:, SLOTBASE[slot] + j] = CH * cid - P * j
        amat_h.append(a)
    ones_k = np.ones((P, 1), np.float16)
    ones_r = np.ones((1, P), np.float32)

    in_maps = []
    for core in range(8):
        b, h = core // 2, core % 2
        xq_cols = np.concatenate(
            [np.arange(c * CH, (c + 1) * CH) for c in CHUNKS_H[h]]
        )
        xqT_b = tile_x(np.ascontiguousarray(xT[b][:, xq_cols]), NSLOT)
        in_maps.append(
            {
                "xT": tile_x(xT[b], 8),
                "xqT": xqT_b,
                "wqT": wqT,
                "wkT": wkT,
                "wvT": wvT,
                "amat": amat_h[h],
                "dmat": dmat,
                "ones_k": ones_k,
                "ones_r": ones_r,
            }
        )
    return in_maps


def _assemble(results):
    out = np.empty((B, S, D), np.float32)
    for core in range(8):
        b, h = core // 2, core % 2
        oT = np.asarray(results[core]["outT"])  # [D, NQ]
        for slot, c in enumerate(CHUNKS_H[h]):
            out[b, c * CH : (c + 1) * CH, :] = oT[:, slot * CH : (slot + 1) * CH].T
    return out


def run(inputs, trace=False, trace_cores=None):
    """Run the kernel; returns (output, BassKernelResults)."""
    from concourse.bass_utils import run_bass_kernel_spmd

    nc = _get_program()
    in_maps = _make_in_maps(
        inputs["x"], inputs["W_query"], inputs["W_key"], inputs["W_value"]
    )
    kw = {}
    if trace:
        kw = dict(trace=True, trace_cores=trace_cores, stitch_traces=False)
    res = run_bass_kernel_spmd(nc, in_maps, list(range(8)), **kw)
    return _assemble(res.results), res


def kernel(x, W_query, W_key, W_value):
    out, _ = run({"x": x, "W_query": W_query, "W_key": W_key, "W_value": W_value})
    return out


# revision 19
# speedup vs baseline: 1.0186x; 1.0080x over previous
"""Causal single-head attention (B=4, S=4096, D=1024) on 8 TRN2 NeuronCores.

Sharding: core = (batch b, half h).  Each core computes attention output for
2048 queries of one batch: query chunks {0,3,4,7} (h=0) or {1,2,5,6} (h=1) of
8x512, which balances causal work.  Each core projects K^T/V for its full
batch (Q projections zippered in between the chunks); K^T lives in SBUF as
four independently-gated fp16 tiles and V is streamed from a DRAM scratch on
the Scalar HWDGE queue.
Scores are computed in the S^T = [k, q] layout so no on-device transposes are
needed anywhere:
  K^T/Q^T/V projections:  psum = sum_d WT[d,:128].T @ x^T[d,:]      (fp16)
  scores^T[k,q]        :  psum = sum_o KT[o,k128].T @ QT[o,q512]    (fp16)
  P = exp(s*scale) * causal_mask   (mask = (iota_k - iota_q) <= a[slot,j])
  den[1,q]             :  ones[k,1].T @ P^T                         (fp16)
  ctx^T[o,q]           :  psum = sum_k V[k,o128].T @ P^T[k,q]       (fp16)
  out = ctx^T * (1/den)  broadcast via ones[1,128].T @ recip[1,q]
"""

import sys

for _p in ("/opt/trn_rl_repo",):
    if _p not in sys.path:
        sys.path.insert(0, _p)

import numpy as np

B, S, D = 4, 4096, 1024
P = 128
CH = 512                       # query chunk
NSLOT = 4                      # chunks per core
NQ = NSLOT * CH                # queries per core
NK = [8, 16, 24, 32]           # k-tiles per slot (uniform across cores)
SLOTBASE = [0, 8, 24, 48]      # amat column base per slot
CHUNKS_H = [[0, 3, 4, 7], [1, 2, 5, 6]]
SCALE = 1.0 / 32.0             # 1/sqrt(D)

_PROGRAM = None


def _build_program():
    import concourse.bass as bass
    import concourse.tile as tile
    import concourse.mybir as mybir
    from concourse import bacc
    from concourse.bass import ds, ts

    f32 = mybir.dt.float32
    f16 = mybir.dt.float16

    nc = bacc.Bacc(trn_type="TRN2", target_bir_lowering=False, debug=False,
                   num_devices=8)

    xT = nc.declare_dram_parameter("xT", [8, P, 8, CH], f16, isOutput=False)
    xqT = nc.declare_dram_parameter("xqT", [NSLOT, P, 8, CH], f16, isOutput=False)
    wqT = nc.declare_dram_parameter("wqT", [P, 8, D], f16, isOutput=False)
    wkT = nc.declare_dram_parameter("wkT", [P, 8, D], f16, isOutput=False)
    wvT = nc.declare_dram_parameter("wvT", [P, 8, D], f16, isOutput=False)
    amat = nc.declare_dram_parameter("amat", [P, 80], f16, isOutput=False)
    dmat = nc.declare_dram_parameter("dmat", [P, CH], f16, isOutput=False)
    ones_k = nc.declare_dram_parameter("ones_k", [P, 1], f16, isOutput=False)
    ones_r = nc.declare_dram_parameter("ones_r", [1, P], f32, isOutput=False)
    outT = nc.declare_dram_parameter("outT", [D, NQ], f32, isOutput=True)

    H = S // 4  # 1024: columns per resident K^T piece
    vscr = nc.dram_tensor("v_scratch", [S, D], f16)

    Exp = mybir.ActivationFunctionType.Exp
    is_le = mybir.AluOpType.is_le
    mult = mybir.AluOpType.mult

    with tile.TileContext(nc, pool_alloc_mode="queue") as tc:
        with (
            tc.tile_pool(name="kt", bufs=1) as kt_pool,
            tc.tile_pool(name="qt", bufs=1) as qt_pool,
            tc.tile_pool(name="const", bufs=1) as const_pool,
        ):
            KTp = [
                kt_pool.tile([P, 8, H], f16, tag=f"kt{i}", name=f"KTp{i}")
                for i in range(4)
            ]
            QTs = [
                qt_pool.tile([P, 8, CH], f16, tag=f"qt{i}", name=f"QTs{i}")
                for i in range(NSLOT)
            ]
            dmat_sb = const_pool.tile([P, CH], f16, tag="dmat")
            amat_sb = const_pool.tile([P, 80], f16, tag="amat")
            ones_k_sb = const_pool.tile([P, 1], f16, tag="onesk")
            ones_r_sb = const_pool.tile([1, P], f32, tag="onesr")
            nc.sync.dma_start(out=dmat_sb[:], in_=dmat[:])
            nc.sync.dma_start(out=amat_sb[:], in_=amat[:])
            nc.sync.dma_start(out=ones_k_sb[:], in_=ones_k[:])
            nc.sync.dma_start(out=ones_r_sb[:], in_=ones_r[:])

            # ---------- Phase 0+1: local projections (K, V, Q zippered) ----
            with (
                tc.tile_pool(name="w0", bufs=1) as w_pool,
                tc.tile_pool(name="xc", bufs=3) as x_pool,
                tc.tile_pool(name="xq", bufs=3) as xq_pool,
                tc.tile_pool(name="vb", bufs=3) as vb_pool,
                tc.tile_pool(name="ps0", bufs=4, space="PSUM") as ps_pool,
            ):
                wk = w_pool.tile([P, 8, D], f16, tag="wk")
                wv = w_pool.tile([P, 8, D], f16, tag="wv")
                wq = w_pool.tile([P, 8, D], f16, tag="wq")
                for half in range(2):
                    nc.sync.dma_start(
                        out=wk[:, :, ds(half * CH, CH)],
                        in_=wkT[:, :, ds(half * CH, CH)],
                    )

                def load_xq(c):
                    xq = xq_pool.tile([P, 8, CH], f16, tag="xq", name=f"xq{c}")
                    nc.scalar.dma_start(
                        out=xq[:],
                        in_=xqT[c],
                    )
                    return xq

                xq_pending = []

                def proj_q(slot):
                    xq = xq_pending[slot]
                    for o in range(8):
                        ps = ps_pool.tile([P, CH], f32, tag="ps", name="psq")
                        for d in range(8):
                            nc.tensor.matmul(
                                ps[:],
                                lhsT=wq[:, d, ts(o, P)],
                                rhs=xq[:, d, :],
                                start=(d == 0),
                                stop=(d == 7),
                            )
                        nc.vector.tensor_copy(QTs[slot][:, o, :], ps[:])

                for c in range(8):
                    xc = x_pool.tile([P, 8, CH], f16, tag="xc", name=f"xc{c}")
                    nc.sync.dma_start(
                        out=xc[:],
                        in_=xT[c],
                    )
                    for o in range(8):
                        ps = ps_pool.tile([P, CH], f32, tag="ps", name="psk")
                        for d in range(8):
                            nc.tensor.matmul(
                                ps[:],
                                lhsT=wk[:, d, ts(o, P)],
                                rhs=xc[:, d, :],
                                start=(d == 0),
                                stop=(d == 7),
                            )
                        nc.vector.tensor_copy(
                            KTp[c // 2][:, o, ds((c % 2) * CH, CH)], ps[:]
                        )
                    if c == 0:
                        # deferred loads: SP/ACT reach these only after the
                        # first chunk's copies, leaving full DMA bandwidth to
                        # the critical wk+xc0 at kernel start
                        nc.sync.dma_start(
                            out=wv[:], in_=wvT[:]
                        )
                        nc.scalar.dma_start(
                            out=wq[:], in_=wqT[:]
                        )
                        xq_pending.append(load_xq(0))
                        xq_pending.append(load_xq(1))
                    for kt_i in range(4):
                        vb = vb_pool.tile([P, D], f16, tag="vb", name="vb")
                        for oh in range(2):
                            ps = ps_pool.tile([P, CH], f32, tag="ps", name="psv")
                            for d in range(8):
                                nc.tensor.matmul(
                                    ps[:],
                                    lhsT=xc[:, d, ts(kt_i, P)],
                                    rhs=wv[:, d, ts(oh, CH)],
                                    start=(d == 0),
                                    stop=(d == 7),
                                )
                            nc.scalar.copy(vb[:, ts(oh, CH)], ps[:])
                        nc.sync.dma_start(
                            out=vscr[ds(c * CH + kt_i * P, P), :], in_=vb[:]
                        )
                    if 1 <= c <= 4:
                        proj_q(c - 1)
                        if c <= 2:
                            xq_pending.append(load_xq(c + 1))

            # ---------------- Phase 2: attention ---------------------------
            with (
                tc.tile_pool(name="ctx", bufs=2) as ctx_pool,
                tc.tile_pool(name="vt", bufs=12) as v_pool,
                tc.tile_pool(name="pt", bufs=12) as p_pool,
                tc.tile_pool(name="et", bufs=3) as e_pool,
                tc.tile_pool(name="fo", bufs=3) as f_pool,
                tc.tile_pool(name="dsb", bufs=2) as den_pool,
                tc.tile_pool(name="pss", bufs=3, space="PSUM") as s_ps_pool,
                tc.tile_pool(name="psc", bufs=3, space="PSUM") as c_ps_pool,
                tc.tile_pool(name="psd", bufs=1, space="PSUM") as d_ps_pool,
                tc.tile_pool(name="psb", bufs=1, space="PSUM") as b_ps_pool,
            ):
                for slot in range(NSLOT):
                    nk = NK[slot]
                    ctx = ctx_pool.tile([P, 8, CH], f32, tag="ctx", name="ctx")
                    den = den_pool.tile([1, CH], f32, tag="den", name="den")
                    for blk in range(nk // 4):
                        p_tiles = []
                        v_tiles = []
                        for j4 in range(4):
                            j = blk * 4 + j4
                            vt = v_pool.tile([P, D], f16, tag="vt", name="vt")
                            nc.scalar.dma_start(out=vt[:], in_=vscr[ds(j * P, P), :])
                            sps = s_ps_pool.tile([P, CH], f32, name="sps")
                            for o in range(8):
                                nc.tensor.matmul(
                                    sps[:],
                                    lhsT=KTp[j // 8][:, o, ds((j % 8) * P, P)],
                                    rhs=QTs[slot][:, o, :],
                                    start=(o == 0),
                                    stop=(o == 7),
                                )
                            et = e_pool.tile([P, CH], f16, tag="et", name="et")
                            nc.scalar.activation(et[:], sps[:], Exp, scale=SCALE)
                            pt = p_pool.tile([P, CH], f16, tag="pt", name="pt")
                            col = SLOTBASE[slot] + j
                            nc.vector.scalar_tensor_tensor(
                                out=pt[:],
                                in0=dmat_sb[:],
                                scalar=amat_sb[:, ds(col, 1)],
                                in1=et[:],
                                op0=is_le,
                                op1=mult,
                            )
                            p_tiles.append(pt)
                            v_tiles.append(vt)
                        dps = d_ps_pool.tile([1, CH], f32, name="dps")
                        for j4 in range(4):
                            nc.tensor.matmul(
                                dps[:],
                                lhsT=ones_k_sb[:],
                                rhs=p_tiles[j4][:],
                                start=(j4 == 0),
                                stop=(j4 == 3),
                            )
                        if blk == 0:
                            nc.vector.tensor_copy(den[:], dps[:])
                        else:
                            nc.vector.tensor_add(den[:], den[:], dps[:])
                        for o in range(8):
                            cps = c_ps_pool.tile([P, CH], f32, name="cps")
                            for j4 in range(4):
                                nc.tensor.matmul(
                                    cps[:],
                                    lhsT=v_tiles[j4][:, ts(o, P)],
                                    rhs=p_tiles[j4][:],
                                    start=(j4 == 0),
                                    stop=(j4 == 3),
                                )
                            if blk == 0:
                                nc.vector.tensor_copy(ctx[:, o, :], cps[:])
                            else:
                                nc.vector.tensor_add(
                                    ctx[:, o, :], ctx[:, o, :], cps[:]
                                )
                    bps = b_ps_pool.tile([P, CH], f32, name="bps")
                    nc.tensor.matmul(
                        bps[:], lhsT=ones_r_sb[:], rhs=den[:], start=True, stop=True
                    )
                    rec = f_pool.tile([P, CH], f32, tag="rec", name="rec")
                    nc.vector.reciprocal(rec[:], bps[:])
                    for o in range(8):
                        ft = f_pool.tile([P, CH], f32, tag="ft", name="ft")
                        nc.vector.tensor_mul(ft[:], ctx[:, o, :], rec[:])
                        nc.sync.dma_start(
                            out=outT[ds(o * P, P), ts(slot, CH)], in_=ft[:]
                        )

    nc.compile()
    return nc


def _get_program():
    global _PROGRAM
    if _PROGRAM is None:
        _PROGRAM = _build_program()
    return _PROGRAM


def _make_in_maps(x, W_query, W_key, W_value):
    xT = np.ascontiguousarray(
        np.asarray(x, dtype=np.float32).transpose(0, 2, 1).astype(np.float16)
    )

    def tile_w(w):
        # [d, o] -> [p, d_slab, o]
        wt = np.asarray(w, dtype=np.float32).T.astype(np.float16)
        return np.ascontiguousarray(wt.reshape(8, P, D).transpose(1, 0, 2))

    def tile_x(xt, nch):
        # [d, s] -> [chunk, p, d_slab, s_off]
        return np.ascontiguousarray(
            xt.reshape(8, P, nch, CH).transpose(2, 1, 0, 3)
        )

    wqT = tile_w(W_query)
    wkT = tile_w(W_key)
    wvT = tile_w(W_value)
    dmat = (
        np.arange(P, dtype=np.float32)[:, None] - np.arange(CH, dtype=np.float32)[None, :]
    )
    dmat = np.ascontiguousarray(dmat.astype(np.float16))
    amat_h = []
    for h in range(2):
        a = np.zeros((P, 80), np.float16)
        for slot in range(NSLOT):
            cid = CHUNKS_H[h][slot]
            for j in range(NK[slot]):
                a[:, SLOTBASE[slot] + j] = CH * cid - P * j
        amat_h.append(a)
    ones_k = np.ones((P, 1), np.float16)
    ones_r = np.ones((1, P), np.float32)

    in_maps = []
    for core in range(8):
        b, h = core // 2, core % 2
        xq_cols = np.concatenate(
            [np.arange(c * CH, (c + 1) * CH) for c in CHUNKS_H[h]]
        )
        xqT_b = tile_x(np.ascontiguousarray(xT[b][:, xq_cols]), NSLOT)
        in_maps.append(
            {
                "xT": tile_x(xT[b], 8),
                "xqT": xqT_b,
                "wqT": wqT,
                "wkT": wkT,
                "wvT": wvT,
                "amat": amat_h[h],
                "dmat": dmat,
                "ones_k": ones_k,
                "ones_r": ones_r,
            }
        )
    return in_maps


def _assemble(results):
    out = np.empty((B, S, D), np.float32)
    for core in range(8):
        b, h = core // 2, core % 2
        oT = np.asarray(results[core]["outT"])  # [D, NQ]
        for slot, c in enumerate(CHUNKS_H[h]):
            out[b, c * CH : (c + 1) * CH, :] = oT[:, slot * CH : (slot + 1) * CH].T
    return out


def run(inputs, trace=False, trace_cores=None):
    """Run the kernel; returns (output, BassKernelResults)."""
    from concourse.bass_utils import run_bass_kernel_spmd

    nc = _get_program()
    in_maps = _make_in_maps(
        inputs["x"], inputs["W_query"], inputs["W_key"], inputs["W_value"]
    )
    kw = {}
    if trace:
        kw = dict(trace=True, trace_cores=trace_cores, stitch_traces=False)
    res = run_bass_kernel_spmd(nc, in_maps, list(range(8)), **kw)
    return _assemble(res.results), res


def kernel(x, W_query, W_key, W_value):
    out, _ = run({"x": x, "W_query": W_query, "W_key": W_key, "W_value": W_value})
    return out
